# revision 25
# baseline (speedup 1.0000x reference)
"""Trainium2 kernel for nn_CenterDisc (segment_reduce).

Computes: per-class (4 classes) mean of x rows (N=4096 rows of 64x512),
then mean pairwise Frobenius distance between the 4 class centers.

Strategy (data-parallel over N, 8 cores):
  - host: cast x to fp8e4m3 (final scalar tolerates it: rel err ~3e-4,
    gate is 2e-2), build one-hot(labels) per shard, shard rows 512/core
  - device: per-class partial sums via TensorE matmul in DoubleRow fp8
    mode (2 contraction rows per PE cell per cycle):
        sums[c, d] = sum_k onehot[k, c] * x[k, d]
    streaming 16 MB/core of fp8 x from HBM, ~1 MB transfers split
    across both HWDGE rings (sync carries s-pair 0, scalar s-pair 1),
    which do nothing else mid-stream. The first compute block is a
    small 1024-column piece with its own quarter-size loads issued
    first, so the pipeline starts early. PSUM->SBUF copies run on
    Vector; output stores go via GpSimd.
  - host: add the 8 partial (4, 32768) fp32 sums, counts =
    bincount(labels), centers + pairwise norms (tiny) on host.
"""

import numpy as np
import ml_dtypes

import concourse.bass as bass
import concourse.tile as tile
from concourse import bacc, mybir
from concourse.bass import ts
from concourse.bass_utils import run_bass_kernel_spmd

# Problem shape (hardcoded per contract)
N, C, PDIM = 4096, 64, 512
D = C * PDIM           # 32768 features per row
NCLS = 4               # num classes
OHW = 16               # one-hot padded width (DoubleRow needs 16B k-step)
CORES = 8
R = N // CORES         # 512 rows per core
KP = 128               # rows per matmul subtile (partition dim)
S = R // KP            # 4 k-subtiles per core
SP = S // 2            # 2 DoubleRow pairs
FB = 1024              # first/last block columns (fast start / short tail)
SB = 4096              # feature columns per regular DMA stripe
MM = 512               # matmul moving free dim (PSUM bank = 512 fp32)
LATE = D - 4096            # blocks past here store via HWDGE rings

_NC_CACHE = None


def _build_bass():
    nc = bacc.Bacc()
    f8 = mybir.dt.float8e4
    x_in = nc.dram_tensor("x", [R, D], f8, kind="ExternalInput")
    oh_in = nc.dram_tensor("onehot", [R, OHW], f8, kind="ExternalInput")
    out = nc.dram_tensor("sums", [NCLS, D], mybir.dt.float32,
                         kind="ExternalOutput")

    # tile[p, s, d] = x[s*128 + p, d]
    x_r = x_in[:, :].rearrange("(s p) d -> p s d", p=KP)     # (128, S, D)
    oh_r = oh_in[:, :].rearrange("(s p) c -> p s c", p=KP)   # (128, S, OHW)

    dr = mybir.MatmulPerfMode.DoubleRow

    # column schedule: uniform 1024-col blocks ([4,2,512] PSUM tiles at
    # bufs=4 fill PSUM exactly and double the matmul/copy decoupling)
    blocks = [(c0, c0 + FB) for c0 in range(0, D, FB)]

    # DMA stripe schedule aligned to block boundaries:
    # 1024, 2048, 7 x 4096, 1024
    stripes = [(0, FB), (FB, FB + 2048)]
    c0 = FB + 2048
    while c0 < D - FB:
        stripes.append((c0, c0 + SB))
        c0 += SB
    stripes.append((D - FB, D))

    with tile.TileContext(nc) as tc:
        with (
            tc.tile_pool(name="ohp", bufs=1) as ohp,
            tc.tile_pool(name="xp", bufs=7) as xp,
            tc.tile_pool(name="outp", bufs=4) as outp,
            tc.tile_pool(name="pp", bufs=4, space="PSUM") as pp,
        ):
            oht = ohp.tile([KP, S, OHW], f8, tag="oh")
            nc.sync.dma_start(out=oht[:], in_=oh_r)

            # ranges[sp] = list of (start, end, tile). Both stripe-0
            # pieces go on the sync ring so the first block gates on a
            # single ring (the scalar ring's first transfer has ~2us
            # extra first-byte latency); scalar meanwhile starts on
            # stripe 1 immediately.
            ranges = {0: [], 1: []}
            for si, (cs, ce) in enumerate(stripes):
                w = ce - cs
                for sp in (1, 0) if si == 0 else (0, 1):
                    tag = f"x{si if si < 2 or si >= len(stripes) - 1 else 'm'}_{sp}"
                    bufs = 8 if w == SB else 1
                    xt = xp.tile([KP, 2, w], f8, tag=tag, bufs=bufs)
                    eng = nc.sync if sp == 0 else nc.scalar
                    eng.dma_start(
                        out=xt[:],
                        in_=x_r[:, 2 * sp:2 * sp + 2, cs:ce])
                    ranges[sp].append((cs, ce, xt))

                # emit compute for every block fully covered by loaded
                # data — but blocks whose stores use the HWDGE rings are
                # held back until every x load has been issued, so a
                # store never precedes a load in a ring's in-order stream
                done = min(r[-1][1] for r in ranges.values())
                limit = done if si == len(stripes) - 1 else min(done, LATE)
                while blocks and blocks[0][1] <= limit:
                    bs, be = blocks.pop(0)
                    _emit_block(nc, out, pp, outp, oht, ranges, bs, be,
                                dr, last=not blocks)
    nc.compile()
    return nc


def _emit_block(nc, out, pp, outp, oht, ranges, bs, be, dr, last):
    js = (be - bs) // MM
    pst = pp.tile([NCLS, js, MM], mybir.dt.float32, tag="ps",
                  name=f"ps{bs}")
    for sp in range(SP):
        for j in range(js):
            c0 = bs + j * MM
            for rs, re, xt in ranges[sp]:
                if rs <= c0 < re:
                    break
            else:
                raise AssertionError(f"no loaded range covers col {c0}")
            nc.tensor.matmul(
                pst[:, j, :],
                oht[:, 2 * sp:2 * sp + 2, 0:NCLS],
                xt[:, :, c0 - rs:c0 - rs + MM],
                start=(sp == 0),
                stop=(sp == SP - 1),
                perf_mode=dr,
            )
    if not last:
        ot = outp.tile([NCLS, be - bs], mybir.dt.float32, tag="ot")
        nc.vector.tensor_copy(out=ot[:], in_=pst[:])
        # late blocks store via the HWDGE rings (their x issues are all
        # behind them by then), so GpSimd's slow SWDGE drain overlaps
        # the stream instead of sitting in the tail
        if bs >= LATE:
            eng = nc.sync if (bs // FB) % 2 == 0 else nc.scalar
        else:
            eng = nc.gpsimd
        eng.dma_start(out=out[:, bs:be], in_=ot[:])
    else:
        # the final block's copies run on Scalar (idle by now, and not
        # behind the big DVE copy of the previous block) and its stores
        # on the idle HWDGE rings, shortening the tail
        h = (be - bs) // 2
        hj = js // 2
        for i, eng in ((0, nc.sync), (1, nc.scalar)):
            oth = outp.tile([NCLS, h], mybir.dt.float32, tag=f"otl{i}")
            nc.scalar.copy(out=oth[:], in_=pst[:, i * hj:(i + 1) * hj, :])
            eng.dma_start(out=out[:, bs + i * h:bs + (i + 1) * h],
                          in_=oth[:])


def _get_nc():
    global _NC_CACHE
    if _NC_CACHE is None:
        _NC_CACHE = _build_bass()
    return _NC_CACHE


def _run(x, labels, trace=False, **spmd_kwargs):
    x = np.asarray(x, dtype=np.float32).reshape(N, D)
    x8 = x.astype(ml_dtypes.float8_e4m3)
    labels = np.asarray(labels).astype(np.int64)
    onehot = np.zeros((N, OHW), dtype=ml_dtypes.float8_e4m3)
    onehot[np.arange(N), labels] = 1.0

    in_maps = [
        {"x": x8[c * R:(c + 1) * R], "onehot": onehot[c * R:(c + 1) * R]}
        for c in range(CORES)
    ]
    nc = _get_nc()
    last_err = None
    for attempt in range(3):
        try:
            br = run_bass_kernel_spmd(nc, in_maps, core_ids=list(range(CORES)),
                                      trace=trace, **spmd_kwargs)
            break
        except Exception as e:  # transient device wedge (NRT_*) — retry
            last_err = e
            import time as _time
            _time.sleep(3.0)
    else:
        raise last_err

    sums = np.zeros((NCLS, D), dtype=np.float64)
    for r in br.results:
        sums += r["sums"].astype(np.float64)
    counts = np.bincount(labels, minlength=NCLS).astype(np.float64)
    safe = np.maximum(counts, 1.0)
    centers = sums / safe[:, None]                         # (NCLS, D)
    diffs = centers[:, None, :] - centers[None, :, :]      # (NCLS, NCLS, D)
    norms = np.sqrt(np.sum(diffs * diffs, axis=-1))        # (NCLS, NCLS)
    iu, ju = np.triu_indices(NCLS, k=1)
    distance = np.sum(norms[iu, ju]) / len(iu)
    return np.asarray(distance, dtype=np.float32), br


def kernel(x, labels):
    result, _ = _run(x, labels, trace=False)
    return result


# revision 26
# speedup vs baseline: 1.0381x; 1.0381x over previous
"""Trainium2 kernel for nn_CenterDisc (segment_reduce).

Computes: per-class (4 classes) mean of x rows (N=4096 rows of 64x512),
then mean pairwise Frobenius distance between the 4 class centers.

Strategy (data-parallel over N, 8 cores):
  - host: cast x to fp8e4m3 (final scalar tolerates it: rel err ~3e-4,
    gate is 2e-2), build one-hot(labels) per shard, shard rows 512/core
  - device: per-class partial sums via TensorE matmul in DoubleRow fp8
    mode (2 contraction rows per PE cell per cycle):
        sums[c, d] = sum_k onehot[k, c] * x[k, d]
    streaming 16 MB/core of fp8 x from HBM, ~1 MB transfers split
    across both HWDGE rings (sync carries s-pair 0, scalar s-pair 1),
    which do nothing else mid-stream. The first compute block is a
    small 1024-column piece with its own quarter-size loads issued
    first, so the pipeline starts early. PSUM->SBUF copies run on
    Vector; output stores go via GpSimd.
  - host: add the 8 partial (4, 32768) fp32 sums, counts =
    bincount(labels), centers + pairwise norms (tiny) on host.
"""

import numpy as np
import ml_dtypes

import concourse.bass as bass
import concourse.tile as tile
from concourse import bacc, mybir
from concourse.bass import ts
from concourse.bass_utils import run_bass_kernel_spmd

# Problem shape (hardcoded per contract)
N, C, PDIM = 4096, 64, 512
D = C * PDIM           # 32768 features per row
NCLS = 4               # num classes
OHW = 16               # one-hot padded width (DoubleRow needs 16B k-step)
CORES = 8
R = N // CORES         # 512 rows per core
KP = 128               # rows per matmul subtile (partition dim)
S = R // KP            # 4 k-subtiles per core
SP = S // 2            # 2 DoubleRow pairs
FB = 1024              # first/last block columns (fast start / short tail)
SB = 4096              # feature columns per regular DMA stripe
MM = 512               # matmul moving free dim (PSUM bank = 512 fp32)
LATE = D - 4 * 2048 - FB   # blocks past here store via HWDGE rings

_NC_CACHE = None


def _build_bass():
    nc = bacc.Bacc()
    f8 = mybir.dt.float8e4
    x_in = nc.dram_tensor("x", [R, D], f8, kind="ExternalInput")
    oh_in = nc.dram_tensor("onehot", [R, OHW], f8, kind="ExternalInput")
    out = nc.dram_tensor("sums", [NCLS, D], mybir.dt.float32,
                         kind="ExternalOutput")

    # tile[p, s, d] = x[s*128 + p, d]
    x_r = x_in[:, :].rearrange("(s p) d -> p s d", p=KP)     # (128, S, D)
    oh_r = oh_in[:, :].rearrange("(s p) c -> p s c", p=KP)   # (128, S, OHW)

    dr = mybir.MatmulPerfMode.DoubleRow

    # column schedule: 1024, 15 x 2048, 1024
    blocks = [(0, FB)]
    c0 = FB
    while c0 < D - FB:
        blocks.append((c0, c0 + 2048))
        c0 += 2048
    blocks.append((D - FB, D))

    # DMA stripe schedule aligned to block boundaries:
    # 1024, 2048, 7 x 4096, 1024
    stripes = [(0, FB), (FB, FB + 2048)]
    c0 = FB + 2048
    while c0 < D - FB:
        stripes.append((c0, c0 + SB))
        c0 += SB
    stripes.append((D - FB, D))

    with tile.TileContext(nc) as tc:
        with (
            tc.tile_pool(name="ohp", bufs=1) as ohp,
            tc.tile_pool(name="xp", bufs=7) as xp,
            tc.tile_pool(name="outp", bufs=4) as outp,
            tc.tile_pool(name="pp", bufs=2, space="PSUM") as pp,
        ):
            oht = ohp.tile([KP, S, OHW], f8, tag="oh")
            nc.sync.dma_start(out=oht[:], in_=oh_r)

            # ranges[sp] = list of (start, end, tile). Both stripe-0
            # pieces go on the sync ring so the first block gates on a
            # single ring (the scalar ring's first transfer has ~2us
            # extra first-byte latency); scalar meanwhile starts on
            # stripe 1 immediately.
            ranges = {0: [], 1: []}
            for si, (cs, ce) in enumerate(stripes):
                w = ce - cs
                for sp in (1, 0) if si == 0 else (0, 1):
                    tag = f"x{si if si < 2 or si >= len(stripes) - 1 else 'm'}_{sp}"
                    bufs = 8 if w == SB else 1
                    xt = xp.tile([KP, 2, w], f8, tag=tag, bufs=bufs)
                    if si == 0:
                        eng = nc.sync
                    else:
                        eng = nc.sync if sp == 0 else nc.scalar
                    eng.dma_start(
                        out=xt[:],
                        in_=x_r[:, 2 * sp:2 * sp + 2, cs:ce])
                    ranges[sp].append((cs, ce, xt))

                # emit compute for every block fully covered by loaded
                # data — but blocks whose stores use the HWDGE rings are
                # held back until every x load has been issued, so a
                # store never precedes a load in a ring's in-order stream
                done = min(r[-1][1] for r in ranges.values())
                limit = done if si == len(stripes) - 1 else min(done, LATE)
                while blocks and blocks[0][1] <= limit:
                    bs, be = blocks.pop(0)
                    _emit_block(nc, out, pp, outp, oht, ranges, bs, be,
                                dr, last=not blocks)
    nc.compile()
    return nc


def _emit_block(nc, out, pp, outp, oht, ranges, bs, be, dr, last):
    js = (be - bs) // MM
    pst = pp.tile([NCLS, js, MM], mybir.dt.float32, tag="ps",
                  name=f"ps{bs}")
    for sp in range(SP):
        for j in range(js):
            c0 = bs + j * MM
            for rs, re, xt in ranges[sp]:
                if rs <= c0 < re:
                    break
            else:
                raise AssertionError(f"no loaded range covers col {c0}")
            nc.tensor.matmul(
                pst[:, j, :],
                oht[:, 2 * sp:2 * sp + 2, 0:NCLS],
                xt[:, :, c0 - rs:c0 - rs + MM],
                start=(sp == 0),
                stop=(sp == SP - 1),
                perf_mode=dr,
            )
    if not last:
        ot = outp.tile([NCLS, be - bs], mybir.dt.float32, tag="ot")
        nc.vector.tensor_copy(out=ot[:], in_=pst[:])
        # late blocks store via the HWDGE rings (their x issues are all
        # behind them by then), so GpSimd's slow SWDGE drain overlaps
        # the stream instead of sitting in the tail
        if bs >= LATE:
            eng = nc.sync if (bs // 2048) % 2 == 0 else nc.scalar
        else:
            eng = nc.gpsimd
        eng.dma_start(out=out[:, bs:be], in_=ot[:])
    else:
        # the final block's copies run on Scalar (idle by now, and not
        # behind the big DVE copy of the previous block) and its stores
        # on the idle HWDGE rings, shortening the tail
        h = (be - bs) // 2
        hj = js // 2
        for i, eng in ((0, nc.sync), (1, nc.scalar)):
            oth = outp.tile([NCLS, h], mybir.dt.float32, tag=f"otl{i}")
            nc.scalar.copy(out=oth[:], in_=pst[:, i * hj:(i + 1) * hj, :])
            eng.dma_start(out=out[:, bs + i * h:bs + (i + 1) * h],
                          in_=oth[:])


def _get_nc():
    global _NC_CACHE
    if _NC_CACHE is None:
        _NC_CACHE = _build_bass()
    return _NC_CACHE


def _run(x, labels, trace=False, **spmd_kwargs):
    x = np.asarray(x, dtype=np.float32).reshape(N, D)
    x8 = x.astype(ml_dtypes.float8_e4m3)
    labels = np.asarray(labels).astype(np.int64)
    onehot = np.zeros((N, OHW), dtype=ml_dtypes.float8_e4m3)
    onehot[np.arange(N), labels] = 1.0

    in_maps = [
        {"x": x8[c * R:(c + 1) * R], "onehot": onehot[c * R:(c + 1) * R]}
        for c in range(CORES)
    ]
    nc = _get_nc()
    last_err = None
    for attempt in range(3):
        try:
            br = run_bass_kernel_spmd(nc, in_maps, core_ids=list(range(CORES)),
                                      trace=trace, **spmd_kwargs)
            break
        except Exception as e:  # transient device wedge (NRT_*) — retry
            last_err = e
            import time as _time
            _time.sleep(3.0)
    else:
        raise last_err

    sums = np.zeros((NCLS, D), dtype=np.float64)
    for r in br.results:
        sums += r["sums"].astype(np.float64)
    counts = np.bincount(labels, minlength=NCLS).astype(np.float64)
    safe = np.maximum(counts, 1.0)
    centers = sums / safe[:, None]                         # (NCLS, D)
    diffs = centers[:, None, :] - centers[None, :, :]      # (NCLS, NCLS, D)
    norms = np.sqrt(np.sum(diffs * diffs, axis=-1))        # (NCLS, NCLS)
    iu, ju = np.triu_indices(NCLS, k=1)
    distance = np.sum(norms[iu, ju]) / len(iu)
    return np.asarray(distance, dtype=np.float32), br


def kernel(x, labels):
    result, _ = _run(x, labels, trace=False)
    return result


# revision 31
# speedup vs baseline: 1.1721x; 1.1291x over previous
"""Trainium2 kernel for nn_CenterDisc (segment_reduce).

Computes: per-class (4 classes) mean of x rows (N=4096 rows of 64x512),
then mean pairwise Frobenius distance between the 4 class centers.

Strategy (data-parallel over N, 8 cores):
  - host: cast x to fp8e4m3 (final scalar tolerates it: rel err ~3e-4,
    gate is 2e-2), build one-hot(labels) per shard, shard rows 512/core
  - device: per-class partial sums via TensorE matmul in DoubleRow fp8
    mode (2 contraction rows per PE cell per cycle):
        sums[c, d] = sum_k onehot[k, c] * x[k, d]
    streaming 16 MB/core of fp8 x from HBM, ~1 MB transfers split
    across both HWDGE rings (sync carries s-pair 0, scalar s-pair 1),
    which do nothing else mid-stream. The first compute block is a
    small 1024-column piece with its own quarter-size loads issued
    first, so the pipeline starts early. PSUM->SBUF copies run on
    Vector; output stores go via GpSimd.
  - host: add the 8 partial (4, 32768) fp32 sums, counts =
    bincount(labels), centers + pairwise norms (tiny) on host.
"""

import numpy as np
import ml_dtypes

import concourse.bass as bass
import concourse.tile as tile
from concourse import bacc, mybir
from concourse.bass import ts
from concourse.bass_utils import run_bass_kernel_spmd

# Problem shape (hardcoded per contract)
N, C, PDIM = 4096, 64, 512
D = C * PDIM           # 32768 features per row
NCLS = 4               # num classes
OHW = 16               # one-hot padded width (DoubleRow needs 16B k-step)
CORES = 8
R = N // CORES         # 512 rows per core
KP = 128               # rows per matmul subtile (partition dim)
S = R // KP            # 4 k-subtiles per core
SP = S // 2            # 2 DoubleRow pairs
FB = 1024              # first/last block columns (fast start / short tail)
SB = 4096              # feature columns per regular DMA stripe
MM = 512               # matmul moving free dim (PSUM bank = 512 fp32)
LATE = D - 4 * 2048 - FB   # blocks past here store via HWDGE rings

_NC_CACHE = None


def _build_bass():
    nc = bacc.Bacc()
    f8 = mybir.dt.float8e4
    x_in = nc.dram_tensor("x", [R, D], f8, kind="ExternalInput")
    oh_in = nc.dram_tensor("onehot", [R, OHW], f8, kind="ExternalInput")
    out = nc.dram_tensor("sums", [NCLS, D], mybir.dt.float32,
                         kind="ExternalOutput")

    # tile[p, s, d] = x[s*128 + p, d]
    x_r = x_in[:, :].rearrange("(s p) d -> p s d", p=KP)     # (128, S, D)
    oh_r = oh_in[:, :].rearrange("(s p) c -> p s c", p=KP)   # (128, S, OHW)

    dr = mybir.MatmulPerfMode.DoubleRow

    # column schedule: 1024, 15 x 2048, 1024
    blocks = [(0, FB)]
    c0 = FB
    while c0 < D - FB:
        blocks.append((c0, c0 + 2048))
        c0 += 2048
    blocks.append((D - FB, D))

    # DMA stripe schedule aligned to block boundaries:
    # 1024, 2048, 7 x 4096, 1024
    stripes = [(0, FB), (FB, FB + 2048)]
    c0 = FB + 2048
    while c0 < D - FB:
        stripes.append((c0, c0 + SB))
        c0 += SB
    stripes.append((D - FB, D))

    with tile.TileContext(nc) as tc:
        with (
            tc.tile_pool(name="ohp", bufs=1) as ohp,
            tc.tile_pool(name="xp", bufs=7) as xp,
            tc.tile_pool(name="outp", bufs=4) as outp,
            tc.tile_pool(name="pp", bufs=2, space="PSUM") as pp,
        ):
            oht = ohp.tile([KP, S, OHW], f8, tag="oh")
            nc.sync.dma_start(out=oht[:], in_=oh_r)

            # ranges[sp] = list of (start, end, tile); stripe 0 is
            # emitted scalar-piece first (that ring has ~2us extra
            # first-byte latency), both rings start streaming at once
            ranges = {0: [], 1: []}
            for si, (cs, ce) in enumerate(stripes):
                w = ce - cs
                for sp in (1, 0) if si == 0 else (0, 1):
                    tag = f"x{si if si < 2 or si >= len(stripes) - 1 else 'm'}_{sp}"
                    bufs = 8 if w == SB else 1
                    xt = xp.tile([KP, 2, w], f8, tag=tag, bufs=bufs)
                    eng = nc.sync if sp == 0 else nc.scalar
                    eng.dma_start(
                        out=xt[:],
                        in_=x_r[:, 2 * sp:2 * sp + 2, cs:ce])
                    ranges[sp].append((cs, ce, xt))

                # emit compute for every block fully covered by loaded
                # data — but blocks whose stores use the HWDGE rings are
                # held back until every x load has been issued, so a
                # store never precedes a load in a ring's in-order stream
                done = min(r[-1][1] for r in ranges.values())
                limit = done if si == len(stripes) - 1 else min(done, LATE)
                while blocks and blocks[0][1] <= limit:
                    bs, be = blocks.pop(0)
                    _emit_block(nc, out, pp, outp, oht, ranges, bs, be,
                                dr, last=not blocks)
    nc.compile()
    return nc


def _emit_block(nc, out, pp, outp, oht, ranges, bs, be, dr, last):
    js = (be - bs) // MM
    pst = pp.tile([NCLS, js, MM], mybir.dt.float32, tag="ps",
                  name=f"ps{bs}")
    for sp in range(SP):
        for j in range(js):
            c0 = bs + j * MM
            for rs, re, xt in ranges[sp]:
                if rs <= c0 < re:
                    break
            else:
                raise AssertionError(f"no loaded range covers col {c0}")
            nc.tensor.matmul(
                pst[:, j, :],
                oht[:, 2 * sp:2 * sp + 2, 0:NCLS],
                xt[:, :, c0 - rs:c0 - rs + MM],
                start=(sp == 0),
                stop=(sp == SP - 1),
                perf_mode=dr,
            )
    if not last:
        ot = outp.tile([NCLS, be - bs], mybir.dt.float32, tag="ot")
        nc.vector.tensor_copy(out=ot[:], in_=pst[:])
        # late blocks store via the HWDGE rings (their x issues are all
        # behind them by then), so GpSimd's slow SWDGE drain overlaps
        # the stream instead of sitting in the tail
        if bs >= LATE:
            eng = nc.sync if (bs // 2048) % 2 == 0 else nc.scalar
        else:
            eng = nc.gpsimd
        eng.dma_start(out=out[:, bs:be], in_=ot[:])
    else:
        # the final block's copies run on Scalar (idle by now, and not
        # behind the big DVE copy of the previous block) and its stores
        # on the idle HWDGE rings, shortening the tail
        h = (be - bs) // 2
        hj = js // 2
        for i, eng in ((0, nc.sync), (1, nc.scalar)):
            oth = outp.tile([NCLS, h], mybir.dt.float32, tag=f"otl{i}")
            nc.scalar.copy(out=oth[:], in_=pst[:, i * hj:(i + 1) * hj, :])
            eng.dma_start(out=out[:, bs + i * h:bs + (i + 1) * h],
                          in_=oth[:])


def _get_nc():
    global _NC_CACHE
    if _NC_CACHE is None:
        _NC_CACHE = _build_bass()
    return _NC_CACHE


def _run(x, labels, trace=False, **spmd_kwargs):
    x = np.asarray(x, dtype=np.float32).reshape(N, D)
    x8 = x.astype(ml_dtypes.float8_e4m3)
    labels = np.asarray(labels).astype(np.int64)
    onehot = np.zeros((N, OHW), dtype=ml_dtypes.float8_e4m3)
    onehot[np.arange(N), labels] = 1.0

    in_maps = [
        {"x": x8[c * R:(c + 1) * R], "onehot": onehot[c * R:(c + 1) * R]}
        for c in range(CORES)
    ]
    nc = _get_nc()
    last_err = None
    for attempt in range(3):
        try:
            br = run_bass_kernel_spmd(nc, in_maps, core_ids=list(range(CORES)),
                                      trace=trace, **spmd_kwargs)
            break
        except Exception as e:  # transient device wedge (NRT_*) — retry
            last_err = e
            import time as _time
            _time.sleep(3.0)
    else:
        raise last_err

    sums = np.zeros((NCLS, D), dtype=np.float64)
    for r in br.results:
        sums += r["sums"].astype(np.float64)
    counts = np.bincount(labels, minlength=NCLS).astype(np.float64)
    safe = np.maximum(counts, 1.0)
    centers = sums / safe[:, None]                         # (NCLS, D)
    diffs = centers[:, None, :] - centers[None, :, :]      # (NCLS, NCLS, D)
    norms = np.sqrt(np.sum(diffs * diffs, axis=-1))        # (NCLS, NCLS)
    iu, ju = np.triu_indices(NCLS, k=1)
    distance = np.sum(norms[iu, ju]) / len(iu)
    return np.asarray(distance, dtype=np.float32), br


def kernel(x, labels):
    result, _ = _run(x, labels, trace=False)
    return result
